# revision 7
# baseline (speedup 1.0000x reference)
"""Trainium2 8-core kernel for MemoryEfficientAttention.

Full multi-head attention layer: Q/K/V projections + exact softmax attention
+ output projection for inputs [B=4, S=2048, D=1024], H=16 heads, dk=64.

Sharding: core c handles batch c//2 and head-half c%2 (8 heads = 512 dims).
Each core produces a partial out-projection [2048, 1024]; the host sums the
two partials per batch and adds the output bias.

Dataflow per core (everything "transposed" so no attention-matrix transposes
are ever needed):
  xT chunks = transpose(x)     PE transpose via identity (fp32), per 512-q group
  QT = Wq.T @ xqT  [512, 2048] (lhsT = Wq natural, rhs = xT chunk) fp32r matmuls
  KT = Wk.T @ xkT  [512, 2048]
  V  = xv @ Wv     [2048, 512] natural (lhsT = xT chunk, rhs = Wv), stored with
                   an extra ones column per head (V_aug [k, 65])
  per head pair (row-packed K=64 matmuls), per q-half:
    sT   = K @ QT              [128k, 4*512] PSUM
    eT   = exp(sT)             ScalarE, PSUM->SBUF  (no max subtraction:
                               scores ~ N(0,1) after the 1/sqrt(dk) folding)
    av  += V_aug.T @ eT        [65, 512] PSUM accumulated over 16 k-tiles;
                               row 64 = softmax denominator
    ocT[h] = av[0:64] * (1/av[64])   DVE reciprocal + DMA row-broadcast,
                               spilled to a DRAM scratch [512, 2048]
  y = ocT.T @ Wo               [2048, 1024] partial, DMA to DRAM
"""

import numpy as np

import concourse.bass as bass
import concourse.mybir as mybir
import concourse.tile as tile
from concourse import bacc
from concourse.masks import make_identity

B, S, D, H, DK = 4, 2048, 1024, 16, 64
NCORES = 8
HPC = H // 2          # heads per core
DH = HPC * DK         # 512 projection dims per core
NJT = DH // 128       # 4 j-tiles (head pairs)
NDT = D // 128        # 8 d-tiles
NQT = S // 128        # 16 q-tiles
NKT = S // 128        # 16 k-tiles
NQG = S // 512        # 4 q-groups
F32 = mybir.dt.float32
F32R = mybir.dt.float32r
EXP = mybir.ActivationFunctionType.Exp


def _r(ap):
    """View an fp32 AP as float32r for full-rate PE matmuls."""
    return ap.bitcast(F32R)


def _bcast_rows(ap_row, nrows):
    """AP that reads one SBUF partition row `nrows` times (partition step 0)."""
    return bass.AP(
        tensor=ap_row.tensor,
        offset=ap_row.offset,
        ap=[[0, nrows]] + [list(x) for x in ap_row.ap[1:]],
    )


def _emit(nc, tc, ctx):
    xq = nc.dram_tensor("xq", [S, D], F32, kind="ExternalInput").ap()
    xk = nc.dram_tensor("xk", [S, D], F32, kind="ExternalInput").ap()
    xv = nc.dram_tensor("xv", [S, D], F32, kind="ExternalInput").ap()
    wq = nc.dram_tensor("wq", [D, DH], F32, kind="ExternalInput").ap()
    wk = nc.dram_tensor("wk", [D, DH], F32, kind="ExternalInput").ap()
    wv = nc.dram_tensor("wv", [D, DH], F32, kind="ExternalInput").ap()
    wo = nc.dram_tensor("wo", [DH, D], F32, kind="ExternalInput").ap()
    bq = nc.dram_tensor("bq", [DH], F32, kind="ExternalInput").ap()
    bk = nc.dram_tensor("bk", [DH], F32, kind="ExternalInput").ap()
    bv = nc.dram_tensor("bv", [DH], F32, kind="ExternalInput").ap()
    y = nc.dram_tensor("y", [S, D], F32, kind="ExternalOutput").ap()

    consts = ctx.enter_context(tc.tile_pool(name="consts", bufs=1))
    wpool = ctx.enter_context(tc.tile_pool(name="weights", bufs=2))
    xstage = ctx.enter_context(tc.tile_pool(name="xstage", bufs=4))
    xtp = ctx.enter_context(tc.tile_pool(name="xtc", bufs=1))
    projp = ctx.enter_context(tc.tile_pool(name="proj", bufs=1))
    expp = ctx.enter_context(tc.tile_pool(name="expt", bufs=2))
    smalls = ctx.enter_context(tc.tile_pool(name="smalls", bufs=2))
    ocstage = ctx.enter_context(tc.tile_pool(name="ocstage", bufs=8))
    ystage = ctx.enter_context(tc.tile_pool(name="ystage", bufs=2))
    dramp = ctx.enter_context(tc.tile_pool(name="drams", bufs=1, space="DRAM"))

    # PSUM: tag "a" = one [128, 2048] (4 banks), tag "b" = [128, 512] x4 (4 banks)
    psum = ctx.enter_context(tc.tile_pool(name="psum", bufs=1, space="PSUM"))

    def pa(name):
        return psum.tile([128, 2048], F32, tag="a", name=name, bufs=1)

    def pb(name):
        return psum.tile([128, 512], F32, tag="b", name=name, bufs=4)

    oc_dram = dramp.tile([DH, S], F32R, name="oc_scratch")

    ident = consts.tile([128, 128], F32)
    make_identity(nc, ident)

    bq_sb = consts.tile([128, NJT], F32)
    nc.sync.dma_start(out=bq_sb, in_=bq.rearrange("(a p) -> p a", p=128))
    bk_sb = consts.tile([128, NJT], F32)
    nc.sync.dma_start(out=bk_sb, in_=bk.rearrange("(a p) -> p a", p=128))
    ones8 = consts.tile([128, HPC], F32)
    nc.vector.memset(ones8, 1.0)
    bv_sb = consts.tile([128, DH], F32)
    nc.sync.dma_start(
        out=bv_sb,
        in_=bass.AP(tensor=bv.tensor, offset=bv.offset, ap=[[0, 128], [1, DH]]),
    )

    qt_t = [projp.tile([128, S], F32R, tag=f"q{jt}", name=f"qT{jt}")
            for jt in range(NJT)]
    kt_t = [projp.tile([128, S], F32R, tag=f"k{jt}", name=f"kT{jt}")
            for jt in range(NJT)]
    v_t = [projp.tile([128, HPC, DK + 1], F32R, tag=f"v{kt}", name=f"v{kt}")
           for kt in range(NKT)]

    def load_project(x_dram, w_dram, name, mode, b_sb, out_t):
        """Stream x, transpose per 512-wide q group, and project.

        mode "T": out_t[jt][:, qg*512:+512] = (x @ W + b).T slice  (QT / KT)
        mode "V": out_t[kt][:, h, 0:64] = (x @ W + b) slice, natural layout
        """
        w_sb = wpool.tile([128, NDT, DH], F32R, tag="w", name=f"w_{name}")
        nc.gpsimd.dma_start(out=w_sb, in_=w_dram.rearrange("(n p) j -> p n j", p=128))
        for qg in range(NQG):
            stg = [xstage.tile([128, D], F32, tag="xstage", name=f"xstg{g}")
                   for g in range(4)]
            for g in range(4):
                qt = qg * 4 + g
                nc.sync.dma_start(out=stg[g], in_=x_dram[qt * 128:(qt + 1) * 128, :])
            xtc = [xtp.tile([128, 512], F32R, tag=f"xtc{dt}", name=f"xtc{dt}")
                   for dt in range(NDT)]
            for dt in range(NDT):
                ptr = pb(f"ptr{dt}")
                for g in range(4):
                    nc.tensor.transpose(
                        ptr[:, g * 128:(g + 1) * 128],
                        stg[g][:, dt * 128:(dt + 1) * 128],
                        ident,
                    )
                nc.vector.tensor_copy(out=xtc[dt][:], in_=ptr[:])
            if mode == "T":
                for jt in range(NJT):
                    pq = pb(f"pq{jt}")
                    for dt in range(NDT):
                        nc.tensor.matmul(
                            pq[:],
                            lhsT=(w_sb[:, dt, jt * 128:(jt + 1) * 128]),
                            rhs=(xtc[dt][:]),
                            start=(dt == 0),
                            stop=(dt == NDT - 1),
                        )
                    nc.vector.tensor_scalar_add(
                        out=out_t[jt][:, qg * 512:(qg + 1) * 512],
                        in0=pq[:],
                        scalar1=b_sb[:, jt:jt + 1],
                    )
            else:
                for ktl in range(4):
                    kt = qg * 4 + ktl
                    pv = pb(f"pv{ktl}")
                    for dt in range(NDT):
                        nc.tensor.matmul(
                            pv[:],
                            lhsT=(xtc[dt][:, ktl * 128:(ktl + 1) * 128]),
                            rhs=(w_sb[:, dt, :]),
                            start=(dt == 0),
                            stop=(dt == NDT - 1),
                        )
                    nc.vector.tensor_copy(out=out_t[kt][:, :, DK], in_=ones8)
                    nc.vector.tensor_add(
                        out=out_t[kt][:, :, 0:DK],
                        in0=pv.rearrange("p (h d) -> p h d", h=HPC),
                        in1=bv_sb.rearrange("p (h d) -> p h d", h=HPC),
                    )

    load_project(xv, wv, "v", "V", bv_sb, v_t)
    load_project(xq, wq, "q", "T", bq_sb, qt_t)
    load_project(xk, wk, "k", "T", bk_sb, kt_t)

    # ---- attention (head pairs jt, q halves qh) ----
    for jt in range(NJT):
        for qh in range(2):
            q0 = qh * 1024
            avs = [pb(f"av{i}") for i in range(4)]
            for kt in range(NKT):
                ps = pa("sT")
                for hh in range(2):
                    r0 = hh * 64
                    for qbh in range(2):
                        nc.tensor.matmul(
                            ps[:, (hh * 2 + qbh) * 512:(hh * 2 + qbh + 1) * 512],
                            lhsT=(kt_t[jt][r0:r0 + 64, kt * 128:(kt + 1) * 128]),
                            rhs=(qt_t[jt][r0:r0 + 64,
                                            q0 + qbh * 512:q0 + (qbh + 1) * 512]),
                            start=True,
                            stop=True,
                        )
                et = expp.tile([128, 2048], F32R, tag="expT", name="expT")
                nc.scalar.activation(et[:], ps[:], EXP)
                for hh in range(2):
                    for qbh in range(2):
                        nc.tensor.matmul(
                            avs[hh * 2 + qbh][0:DK + 1, :],
                            lhsT=(v_t[kt][:, 2 * jt + hh, :]),
                            rhs=(et[:, (hh * 2 + qbh) * 512:(hh * 2 + qbh + 1) * 512]),
                            start=(kt == 0),
                            stop=(kt == NKT - 1),
                        )
            for hh in range(2):
                for qbh in range(2):
                    av = avs[hh * 2 + qbh]
                    qoff = q0 + qbh * 512
                    rc = smalls.tile([128, 512], F32, tag="rcp", name="rcp")
                    nc.vector.reciprocal(rc[DK:DK + 1, :], av[DK:DK + 1, :])
                    rdram = dramp.tile([512], F32, tag="rdram", name="rdram",
                                       bufs=4)
                    nc.sync.dma_start(out=rdram[None, :], in_=rc[DK:DK + 1, :])
                    rb = smalls.tile([128, 512], F32, tag="rbc", name="rbc")
                    nc.sync.dma_start(out=rb[0:DK, :],
                                      in_=_bcast_rows(rdram[None, :], DK))
                    op = smalls.tile([128, 512], F32R, tag="ocp", name="ocp")
                    nc.vector.tensor_mul(out=op[0:DK, :], in0=av[0:DK, :],
                                         in1=rb[0:DK, :])
                    nc.sync.dma_start(
                        out=oc_dram[jt * 128 + hh * 64:jt * 128 + (hh + 1) * 64,
                                    qoff:qoff + 512],
                        in_=op[0:DK, :],
                    )

    # ---- output projection (partial y; host adds the pair + bias) ----
    wo_sb = wpool.tile([128, NJT, D], F32R, tag="w", name="w_o")
    nc.gpsimd.dma_start(out=wo_sb, in_=wo.rearrange("(n p) j -> p n j", p=128))
    for qt in range(NQT):
        ocl = [ocstage.tile([128, 128], F32R, tag="ocl", name=f"ocl{jt}")
               for jt in range(NJT)]
        for jt in range(NJT):
            nc.sync.dma_start(
                out=ocl[jt],
                in_=oc_dram[jt * 128:(jt + 1) * 128, qt * 128:(qt + 1) * 128],
            )
        py = [pb(f"py{nb}") for nb in range(2)]
        for jt in range(NJT):
            for nb in range(2):
                nc.tensor.matmul(
                    py[nb][:],
                    lhsT=(ocl[jt][:]),
                    rhs=(wo_sb[:, jt, nb * 512:(nb + 1) * 512]),
                    start=(jt == 0),
                    stop=(jt == NJT - 1),
                )
        ys = ystage.tile([128, D], F32, tag="y", name="ys")
        for nb in range(2):
            nc.vector.tensor_copy(out=ys[:, nb * 512:(nb + 1) * 512], in_=py[nb][:])
        nc.sync.dma_start(out=y[qt * 128:(qt + 1) * 128, :], in_=ys[:])


_CACHE = {}


def _build():
    if "nc" in _CACHE:
        return _CACHE["nc"]
    from contextlib import ExitStack

    nc = bacc.Bacc("TRN2", target_bir_lowering=False, debug=False,
                   num_devices=NCORES)
    with tile.TileContext(nc) as tc:
        with ExitStack() as ctx:
            _emit(nc, tc, ctx)
    nc.compile()
    _CACHE["nc"] = nc
    return nc


def make_in_maps(query, key, value, Wq, bq, Wk, bk, Wv, bv, Wo, bo):
    arrs = [np.ascontiguousarray(np.asarray(a, dtype=np.float32))
            for a in (query, key, value, Wq, bq, Wk, bk, Wv, bv, Wo, bo)]
    query, key, value, Wq, bq, Wk, bk, Wv, bv, Wo, bo = arrs
    scale = np.float32(1.0 / np.sqrt(DK))
    in_maps = []
    for c in range(NCORES):
        b, hh = divmod(c, 2)
        js = slice(hh * DH, (hh + 1) * DH)
        in_maps.append({
            "xq": query[b],
            "xk": key[b],
            "xv": value[b],
            "wq": np.ascontiguousarray(Wq[:, js] * scale),
            "bq": np.ascontiguousarray(bq[js] * scale),
            "wk": np.ascontiguousarray(Wk[:, js]),
            "bk": np.ascontiguousarray(bk[js]),
            "wv": np.ascontiguousarray(Wv[:, js]),
            "bv": np.ascontiguousarray(bv[js]),
            "wo": np.ascontiguousarray(Wo[js, :]),
        })
    return in_maps


LAST_RESULTS = None


def kernel(query, key, value, Wq, bq, Wk, bk, Wv, bv, Wo, bo):
    global LAST_RESULTS
    import os
    from concourse.bass_utils import run_bass_kernel_spmd

    nc = _build()
    in_maps = make_in_maps(query, key, value, Wq, bq, Wk, bk, Wv, bv, Wo, bo)
    trace = bool(int(os.environ.get("KERNEL_TRACE", "0")))
    res = run_bass_kernel_spmd(nc, in_maps, list(range(NCORES)), trace=trace)
    LAST_RESULTS = res
    bo32 = np.asarray(bo, dtype=np.float32)
    out = np.empty((B, S, D), dtype=np.float32)
    for b in range(B):
        out[b] = res.results[2 * b]["y"] + res.results[2 * b + 1]["y"] + bo32
    return out


# revision 11
# speedup vs baseline: 1.1145x; 1.1145x over previous
"""Trainium2 8-core kernel for MemoryEfficientAttention.

Full multi-head attention layer: Q/K/V projections + exact softmax attention
+ output projection for inputs [B=4, S=2048, D=1024], H=16 heads, dk=64.

Sharding: core c handles batch c//2 and head-half c%2 (8 heads = 512 dims).
Each core produces a partial out-projection [2048, 1024]; the host sums the
two partials per batch and adds the output bias.

Dataflow per core (everything "transposed" so no attention-matrix transposes
are ever needed):
  xT chunks = transpose(x)     PE transpose via identity (fp32), per 512-q group
  QT = Wq.T @ xqT  [512, 2048] (lhsT = Wq natural, rhs = xT chunk) fp32r matmuls
  KT = Wk.T @ xkT  [512, 2048]
  V  = xv @ Wv     [2048, 512] natural (lhsT = xT chunk, rhs = Wv), stored with
                   an extra ones column per head (V_aug [k, 65])
  per head pair (row-packed K=64 matmuls), per q-half:
    sT   = K @ QT              [128k, 4*512] PSUM
    eT   = exp(sT)             ScalarE, PSUM->SBUF  (no max subtraction:
                               scores ~ N(0,1) after the 1/sqrt(dk) folding)
    av  += V_aug.T @ eT        [65, 512] PSUM accumulated over 16 k-tiles;
                               row 64 = softmax denominator
    ocT[h] = av[0:64] * (1/av[64])   DVE reciprocal + DMA row-broadcast,
                               spilled to a DRAM scratch [512, 2048]
  y = ocT.T @ Wo               [2048, 1024] partial, DMA to DRAM
"""

import numpy as np

import concourse.bass as bass
import concourse.mybir as mybir
import concourse.tile as tile
from concourse import bacc
from concourse.masks import make_identity

B, S, D, H, DK = 4, 2048, 1024, 16, 64
NCORES = 8
HPC = H // 2          # heads per core
DH = HPC * DK         # 512 projection dims per core
NJT = DH // 128       # 4 j-tiles (head pairs)
NDT = D // 128        # 8 d-tiles
NQT = S // 128        # 16 q-tiles
NKT = S // 128        # 16 k-tiles
NQG = S // 512        # 4 q-groups
F32 = mybir.dt.float32
F32R = mybir.dt.float32r
EXP = mybir.ActivationFunctionType.Exp


def _r(ap):
    """View an fp32 AP as float32r for full-rate PE matmuls."""
    return ap.bitcast(F32R)


def _bcast_rows(ap_row, nrows):
    """AP that reads one SBUF partition row `nrows` times (partition step 0)."""
    return bass.AP(
        tensor=ap_row.tensor,
        offset=ap_row.offset,
        ap=[[0, nrows]] + [list(x) for x in ap_row.ap[1:]],
    )


def _emit(nc, tc, ctx):
    xq = nc.dram_tensor("xq", [S, D], F32, kind="ExternalInput").ap()
    xk = nc.dram_tensor("xk", [S, D], F32, kind="ExternalInput").ap()
    xv = nc.dram_tensor("xv", [S, D], F32, kind="ExternalInput").ap()
    wq = nc.dram_tensor("wq", [D, DH], F32, kind="ExternalInput").ap()
    wk = nc.dram_tensor("wk", [D, DH], F32, kind="ExternalInput").ap()
    wv = nc.dram_tensor("wv", [D, DH], F32, kind="ExternalInput").ap()
    wo = nc.dram_tensor("wo", [DH, D], F32, kind="ExternalInput").ap()
    bq = nc.dram_tensor("bq", [DH], F32, kind="ExternalInput").ap()
    bk = nc.dram_tensor("bk", [DH], F32, kind="ExternalInput").ap()
    bv = nc.dram_tensor("bv", [DH], F32, kind="ExternalInput").ap()
    y = nc.dram_tensor("y", [S, D], F32, kind="ExternalOutput").ap()

    consts = ctx.enter_context(tc.tile_pool(name="consts", bufs=1))
    wpool = ctx.enter_context(tc.tile_pool(name="weights", bufs=2))
    xstage = ctx.enter_context(tc.tile_pool(name="xstage", bufs=4))
    xtp = ctx.enter_context(tc.tile_pool(name="xtc", bufs=1))
    projp = ctx.enter_context(tc.tile_pool(name="proj", bufs=1))
    expp = ctx.enter_context(tc.tile_pool(name="expt", bufs=2))
    smalls = ctx.enter_context(tc.tile_pool(name="smalls", bufs=2))
    ocstage = ctx.enter_context(tc.tile_pool(name="ocstage", bufs=8))
    ystage = ctx.enter_context(tc.tile_pool(name="ystage", bufs=2))
    dramp = ctx.enter_context(tc.tile_pool(name="drams", bufs=1, space="DRAM"))

    # PSUM: tag "s" = [128, 1024] x2 (4 banks), tag "b" = [128, 512] x4 (4 banks)
    psum = ctx.enter_context(tc.tile_pool(name="psum", bufs=1, space="PSUM"))

    def ps_tile(name):
        return psum.tile([128, 1024], F32, tag="s", name=name, bufs=2)

    def pb(name):
        return psum.tile([128, 512], F32, tag="b", name=name, bufs=4)

    oc_dram = dramp.tile([DH, S], F32R, name="oc_scratch")

    ident = consts.tile([128, 128], F32)
    make_identity(nc, ident)

    bq_sb = consts.tile([128, NJT], F32)
    nc.sync.dma_start(out=bq_sb, in_=bq.rearrange("(a p) -> p a", p=128))
    bk_sb = consts.tile([128, NJT], F32)
    nc.sync.dma_start(out=bk_sb, in_=bk.rearrange("(a p) -> p a", p=128))
    ones8 = consts.tile([128, HPC], F32)
    nc.vector.memset(ones8, 1.0)
    bv_sb = consts.tile([128, DH], F32)
    nc.sync.dma_start(
        out=bv_sb,
        in_=bass.AP(tensor=bv.tensor, offset=bv.offset, ap=[[0, 128], [1, DH]]),
    )

    qt_t = [projp.tile([128, S], F32R, tag=f"q{jt}", name=f"qT{jt}")
            for jt in range(NJT)]
    kt_t = [projp.tile([128, S], F32R, tag=f"k{jt}", name=f"kT{jt}")
            for jt in range(NJT)]
    v_t = [projp.tile([128, HPC, DK + 1], F32R, tag=f"v{kt}", name=f"v{kt}")
           for kt in range(NKT)]

    def load_project(x_dram, w_dram, name, mode, b_sb, out_t):
        """Stream x, transpose per 512-wide q group, and project.

        mode "T": out_t[jt][:, qg*512:+512] = (x @ W + b).T slice  (QT / KT)
        mode "V": out_t[kt][:, h, 0:64] = (x @ W + b) slice, natural layout
        """
        w_sb = wpool.tile([128, NDT, DH], F32R, tag="w", name=f"w_{name}")
        nc.gpsimd.dma_start(out=w_sb, in_=w_dram.rearrange("(n p) j -> p n j", p=128))
        for qg in range(NQG):
            stg = [xstage.tile([128, D], F32, tag="xstage", name=f"xstg{g}")
                   for g in range(4)]
            for g in range(4):
                qt = qg * 4 + g
                nc.sync.dma_start(out=stg[g], in_=x_dram[qt * 128:(qt + 1) * 128, :])
            xtc = [xtp.tile([128, 512], F32R, tag=f"xtc{dt}", name=f"xtc{dt}")
                   for dt in range(NDT)]
            for dt in range(NDT):
                ptr = pb(f"ptr{dt}")
                for g in range(4):
                    nc.tensor.transpose(
                        ptr[:, g * 128:(g + 1) * 128],
                        stg[g][:, dt * 128:(dt + 1) * 128],
                        ident,
                    )
                nc.vector.tensor_copy(out=xtc[dt][:], in_=ptr[:])
            if mode == "T":
                for jt in range(NJT):
                    pq = pb(f"pq{jt}")
                    for dt in range(NDT):
                        nc.tensor.matmul(
                            pq[:],
                            lhsT=(w_sb[:, dt, jt * 128:(jt + 1) * 128]),
                            rhs=(xtc[dt][:]),
                            start=(dt == 0),
                            stop=(dt == NDT - 1),
                        )
                    nc.vector.tensor_scalar_add(
                        out=out_t[jt][:, qg * 512:(qg + 1) * 512],
                        in0=pq[:],
                        scalar1=b_sb[:, jt:jt + 1],
                    )
            else:
                for ktl in range(4):
                    kt = qg * 4 + ktl
                    pv = pb(f"pv{ktl}")
                    for dt in range(NDT):
                        nc.tensor.matmul(
                            pv[:],
                            lhsT=(xtc[dt][:, ktl * 128:(ktl + 1) * 128]),
                            rhs=(w_sb[:, dt, :]),
                            start=(dt == 0),
                            stop=(dt == NDT - 1),
                        )
                    nc.vector.tensor_copy(out=out_t[kt][:, :, DK], in_=ones8)
                    nc.vector.tensor_add(
                        out=out_t[kt][:, :, 0:DK],
                        in0=pv.rearrange("p (h d) -> p h d", h=HPC),
                        in1=bv_sb.rearrange("p (h d) -> p h d", h=HPC),
                    )

    load_project(xv, wv, "v", "V", bv_sb, v_t)
    load_project(xq, wq, "q", "T", bq_sb, qt_t)
    load_project(xk, wk, "k", "T", bk_sb, kt_t)

    # ---- attention (head pairs jt, q halves qh) ----
    for jt in range(NJT):
        for qh in range(2):
            q0 = qh * 1024
            avs = [pb(f"av{i}") for i in range(4)]
            for kt in range(NKT):
                for hh in range(2):
                    r0 = hh * 64
                    ps = ps_tile(f"sT{hh}")
                    for qbh in range(2):
                        nc.tensor.matmul(
                            ps[:, qbh * 512:(qbh + 1) * 512],
                            lhsT=(kt_t[jt][r0:r0 + 64, kt * 128:(kt + 1) * 128]),
                            rhs=(qt_t[jt][r0:r0 + 64,
                                          q0 + qbh * 512:q0 + (qbh + 1) * 512]),
                            start=True,
                            stop=True,
                        )
                    et = expp.tile([128, 1024], F32R, tag="expT", name="expT")
                    nc.scalar.activation(et[:], ps[:], EXP)
                    for qbh in range(2):
                        nc.tensor.matmul(
                            avs[hh * 2 + qbh][0:DK + 1, :],
                            lhsT=(v_t[kt][:, 2 * jt + hh, :]),
                            rhs=(et[:, qbh * 512:(qbh + 1) * 512]),
                            start=(kt == 0),
                            stop=(kt == NKT - 1),
                        )
            for hh in range(2):
                for qbh in range(2):
                    av = avs[hh * 2 + qbh]
                    qoff = q0 + qbh * 512
                    rc = smalls.tile([128, 512], F32, tag="rcp", name="rcp")
                    nc.vector.reciprocal(rc[DK:DK + 1, :], av[DK:DK + 1, :])
                    rdram = dramp.tile([512], F32, tag="rdram", name="rdram",
                                       bufs=4)
                    nc.sync.dma_start(out=rdram[None, :], in_=rc[DK:DK + 1, :])
                    rb = smalls.tile([128, 512], F32, tag="rbc", name="rbc")
                    nc.sync.dma_start(out=rb[0:DK, :],
                                      in_=_bcast_rows(rdram[None, :], DK))
                    op = smalls.tile([128, 512], F32R, tag="ocp", name="ocp")
                    nc.vector.tensor_mul(out=op[0:DK, :], in0=av[0:DK, :],
                                         in1=rb[0:DK, :])
                    nc.sync.dma_start(
                        out=oc_dram[jt * 128 + hh * 64:jt * 128 + (hh + 1) * 64,
                                    qoff:qoff + 512],
                        in_=op[0:DK, :],
                    )

    # ---- output projection (partial y; host adds the pair + bias) ----
    wo_sb = wpool.tile([128, NJT, D], F32R, tag="w", name="w_o")
    nc.gpsimd.dma_start(out=wo_sb, in_=wo.rearrange("(n p) j -> p n j", p=128))
    for qt in range(NQT):
        ocl = [ocstage.tile([128, 128], F32R, tag="ocl", name=f"ocl{jt}")
               for jt in range(NJT)]
        for jt in range(NJT):
            nc.sync.dma_start(
                out=ocl[jt],
                in_=oc_dram[jt * 128:(jt + 1) * 128, qt * 128:(qt + 1) * 128],
            )
        py = [pb(f"py{nb}") for nb in range(2)]
        for jt in range(NJT):
            for nb in range(2):
                nc.tensor.matmul(
                    py[nb][:],
                    lhsT=(ocl[jt][:]),
                    rhs=(wo_sb[:, jt, nb * 512:(nb + 1) * 512]),
                    start=(jt == 0),
                    stop=(jt == NJT - 1),
                )
        ys = ystage.tile([128, D], F32, tag="y", name="ys")
        for nb in range(2):
            nc.vector.tensor_copy(out=ys[:, nb * 512:(nb + 1) * 512], in_=py[nb][:])
        nc.sync.dma_start(out=y[qt * 128:(qt + 1) * 128, :], in_=ys[:])


_CACHE = {}


def _build():
    if "nc" in _CACHE:
        return _CACHE["nc"]
    from contextlib import ExitStack

    nc = bacc.Bacc("TRN2", target_bir_lowering=False, debug=False,
                   num_devices=NCORES)
    with tile.TileContext(nc) as tc:
        with ExitStack() as ctx:
            _emit(nc, tc, ctx)
    nc.compile()
    _CACHE["nc"] = nc
    return nc


def make_in_maps(query, key, value, Wq, bq, Wk, bk, Wv, bv, Wo, bo):
    arrs = [np.ascontiguousarray(np.asarray(a, dtype=np.float32))
            for a in (query, key, value, Wq, bq, Wk, bk, Wv, bv, Wo, bo)]
    query, key, value, Wq, bq, Wk, bk, Wv, bv, Wo, bo = arrs
    scale = np.float32(1.0 / np.sqrt(DK))
    in_maps = []
    for c in range(NCORES):
        b, hh = divmod(c, 2)
        js = slice(hh * DH, (hh + 1) * DH)
        in_maps.append({
            "xq": query[b],
            "xk": key[b],
            "xv": value[b],
            "wq": np.ascontiguousarray(Wq[:, js] * scale),
            "bq": np.ascontiguousarray(bq[js] * scale),
            "wk": np.ascontiguousarray(Wk[:, js]),
            "bk": np.ascontiguousarray(bk[js]),
            "wv": np.ascontiguousarray(Wv[:, js]),
            "bv": np.ascontiguousarray(bv[js]),
            "wo": np.ascontiguousarray(Wo[js, :]),
        })
    return in_maps


LAST_RESULTS = None


def kernel(query, key, value, Wq, bq, Wk, bk, Wv, bv, Wo, bo):
    global LAST_RESULTS
    import os
    from concourse.bass_utils import run_bass_kernel_spmd

    nc = _build()
    in_maps = make_in_maps(query, key, value, Wq, bq, Wk, bk, Wv, bv, Wo, bo)
    trace = bool(int(os.environ.get("KERNEL_TRACE", "0")))
    res = run_bass_kernel_spmd(nc, in_maps, list(range(NCORES)), trace=trace)
    LAST_RESULTS = res
    bo32 = np.asarray(bo, dtype=np.float32)
    out = np.empty((B, S, D), dtype=np.float32)
    for b in range(B):
        out[b] = res.results[2 * b]["y"] + res.results[2 * b + 1]["y"] + bo32
    return out


# revision 13
# speedup vs baseline: 1.1906x; 1.0683x over previous
"""Trainium2 8-core kernel for MemoryEfficientAttention.

Full multi-head attention layer: Q/K/V projections + exact softmax attention
+ output projection for inputs [B=4, S=2048, D=1024], H=16 heads, dk=64.

Sharding: core c handles batch c//2 and head-half c%2 (8 heads = 512 dims).
Each core produces a partial out-projection [2048, 1024]; the host sums the
two partials per batch and adds the output bias.

Dataflow per core (everything "transposed" so no attention-matrix transposes
are ever needed):
  xT chunks = transpose(x)     PE transpose via identity (fp32), per 512-q group
  QT = Wq.T @ xqT  [512, 2048] (lhsT = Wq natural, rhs = xT chunk) fp32r matmuls
  KT = Wk.T @ xkT  [512, 2048]
  V  = xv @ Wv     [2048, 512] natural (lhsT = xT chunk, rhs = Wv), stored with
                   an extra ones column per head (V_aug [k, 65])
  per head pair (row-packed K=64 matmuls), per q-half:
    sT   = K @ QT              [128k, 4*512] PSUM
    eT   = exp(sT)             ScalarE, PSUM->SBUF  (no max subtraction:
                               scores ~ N(0,1) after the 1/sqrt(dk) folding)
    av  += V_aug.T @ eT        [65, 512] PSUM accumulated over 16 k-tiles;
                               row 64 = softmax denominator
    ocT[h] = av[0:64] * (1/av[64])   DVE reciprocal + DMA row-broadcast,
                               spilled to a DRAM scratch [512, 2048]
  y = ocT.T @ Wo               [2048, 1024] partial, DMA to DRAM
"""

import numpy as np

import concourse.bass as bass
import concourse.mybir as mybir
import concourse.tile as tile
from concourse import bacc
from concourse.masks import make_identity

B, S, D, H, DK = 4, 2048, 1024, 16, 64
NCORES = 8
HPC = H // 2          # heads per core
DH = HPC * DK         # 512 projection dims per core
NJT = DH // 128       # 4 j-tiles (head pairs)
NDT = D // 128        # 8 d-tiles
NQT = S // 128        # 16 q-tiles
NKT = S // 128        # 16 k-tiles
NQG = S // 512        # 4 q-groups
F32 = mybir.dt.float32
F32R = mybir.dt.float32r
EXP = mybir.ActivationFunctionType.Exp


def _r(ap):
    """View an fp32 AP as float32r for full-rate PE matmuls."""
    return ap.bitcast(F32R)


def _bcast_rows(ap_row, nrows):
    """AP that reads one SBUF partition row `nrows` times (partition step 0)."""
    return bass.AP(
        tensor=ap_row.tensor,
        offset=ap_row.offset,
        ap=[[0, nrows]] + [list(x) for x in ap_row.ap[1:]],
    )


def _emit(nc, tc, ctx):
    xq = nc.dram_tensor("xq", [S, D], F32, kind="ExternalInput").ap()
    xk = nc.dram_tensor("xk", [S, D], F32, kind="ExternalInput").ap()
    xv = nc.dram_tensor("xv", [S, D], F32, kind="ExternalInput").ap()
    wq = nc.dram_tensor("wq", [D, DH], F32, kind="ExternalInput").ap()
    wk = nc.dram_tensor("wk", [D, DH], F32, kind="ExternalInput").ap()
    wv = nc.dram_tensor("wv", [D, DH], F32, kind="ExternalInput").ap()
    wo = nc.dram_tensor("wo", [DH, D], F32, kind="ExternalInput").ap()
    bq = nc.dram_tensor("bq", [DH], F32, kind="ExternalInput").ap()
    bk = nc.dram_tensor("bk", [DH], F32, kind="ExternalInput").ap()
    bv = nc.dram_tensor("bv", [DH], F32, kind="ExternalInput").ap()
    y = nc.dram_tensor("y", [S, D], F32, kind="ExternalOutput").ap()

    consts = ctx.enter_context(tc.tile_pool(name="consts", bufs=1))
    wpool = ctx.enter_context(tc.tile_pool(name="weights", bufs=2))
    xstage = ctx.enter_context(tc.tile_pool(name="xstage", bufs=4))
    xtp = ctx.enter_context(tc.tile_pool(name="xtc", bufs=1))
    projp = ctx.enter_context(tc.tile_pool(name="proj", bufs=1))
    expp = ctx.enter_context(tc.tile_pool(name="expt", bufs=2))
    smalls = ctx.enter_context(tc.tile_pool(name="smalls", bufs=2))
    ocstage = ctx.enter_context(tc.tile_pool(name="ocstage", bufs=8))
    ystage = ctx.enter_context(tc.tile_pool(name="ystage", bufs=2))
    dramp = ctx.enter_context(tc.tile_pool(name="drams", bufs=1, space="DRAM"))

    # PSUM: tag "s" = [128, 1024] x2 (4 banks), tag "b" = [128, 512] x4 (4 banks)
    psum = ctx.enter_context(tc.tile_pool(name="psum", bufs=1, space="PSUM"))

    def ps_tile(name):
        return psum.tile([128, 1024], F32, tag="s", name=name, bufs=2)

    def pb(name):
        return psum.tile([128, 512], F32, tag="b", name=name, bufs=4)

    oc_dram = dramp.tile([DH, S], F32R, name="oc_scratch")

    ident = consts.tile([128, 128], F32)
    make_identity(nc, ident)

    bq_sb = consts.tile([128, NJT], F32)
    nc.sync.dma_start(out=bq_sb, in_=bq.rearrange("(a p) -> p a", p=128))
    bk_sb = consts.tile([128, NJT], F32)
    nc.sync.dma_start(out=bk_sb, in_=bk.rearrange("(a p) -> p a", p=128))
    ones8 = consts.tile([128, HPC], F32)
    nc.vector.memset(ones8, 1.0)
    bv_sb = consts.tile([128, DH], F32)
    nc.sync.dma_start(
        out=bv_sb,
        in_=bass.AP(tensor=bv.tensor, offset=bv.offset, ap=[[0, 128], [1, DH]]),
    )

    qt_t = [projp.tile([128, S], F32R, tag=f"q{jt}", name=f"qT{jt}")
            for jt in range(NJT)]
    kt_t = [projp.tile([128, S], F32R, tag=f"k{jt}", name=f"kT{jt}")
            for jt in range(NJT)]
    v_t = [projp.tile([128, HPC, DK + 1], F32R, tag=f"v{kt}", name=f"v{kt}")
           for kt in range(NKT)]

    def load_project(x_dram, w_dram, name, mode, b_sb, out_t):
        """Stream x, transpose per 512-wide q group, and project.

        mode "T": out_t[jt][:, qg*512:+512] = (x @ W + b).T slice  (QT / KT)
        mode "V": out_t[kt][:, h, 0:64] = (x @ W + b) slice, natural layout
        """
        w_sb = wpool.tile([128, NDT, DH], F32R, tag="w", name=f"w_{name}")
        nc.gpsimd.dma_start(out=w_sb, in_=w_dram.rearrange("(n p) j -> p n j", p=128))
        for qg in range(NQG):
            stg = [xstage.tile([128, D], F32, tag="xstage", name=f"xstg{g}")
                   for g in range(4)]
            for g in range(4):
                qt = qg * 4 + g
                nc.sync.dma_start(out=stg[g], in_=x_dram[qt * 128:(qt + 1) * 128, :])
            xtc = [xtp.tile([128, 512], F32R, tag=f"xtc{dt}", name=f"xtc{dt}")
                   for dt in range(NDT)]
            for dt in range(NDT):
                ptr = pb(f"ptr{dt}")
                for g in range(4):
                    nc.tensor.transpose(
                        ptr[:, g * 128:(g + 1) * 128],
                        stg[g][:, dt * 128:(dt + 1) * 128],
                        ident,
                    )
                nc.vector.tensor_copy(out=xtc[dt][:], in_=ptr[:])
            if mode == "T":
                for jt in range(NJT):
                    pq = pb(f"pq{jt}")
                    for dt in range(NDT):
                        nc.tensor.matmul(
                            pq[:],
                            lhsT=(w_sb[:, dt, jt * 128:(jt + 1) * 128]),
                            rhs=(xtc[dt][:]),
                            start=(dt == 0),
                            stop=(dt == NDT - 1),
                        )
                    nc.vector.tensor_scalar_add(
                        out=out_t[jt][:, qg * 512:(qg + 1) * 512],
                        in0=pq[:],
                        scalar1=b_sb[:, jt:jt + 1],
                    )
            else:
                for ktl in range(4):
                    kt = qg * 4 + ktl
                    pv = pb(f"pv{ktl}")
                    for dt in range(NDT):
                        nc.tensor.matmul(
                            pv[:],
                            lhsT=(xtc[dt][:, ktl * 128:(ktl + 1) * 128]),
                            rhs=(w_sb[:, dt, :]),
                            start=(dt == 0),
                            stop=(dt == NDT - 1),
                        )
                    nc.vector.tensor_copy(out=out_t[kt][:, :, DK], in_=ones8)
                    nc.vector.tensor_add(
                        out=out_t[kt][:, :, 0:DK],
                        in0=pv.rearrange("p (h d) -> p h d", h=HPC),
                        in1=bv_sb.rearrange("p (h d) -> p h d", h=HPC),
                    )

    load_project(xv, wv, "v", "V", bv_sb, v_t)
    load_project(xq, wq, "q", "T", bq_sb, qt_t)
    load_project(xk, wk, "k", "T", bk_sb, kt_t)

    # ---- attention (head pairs jt, q halves qh) ----
    for jt in range(NJT):
        for qh in range(2):
            q0 = qh * 1024
            avs = [pb(f"av{i}") for i in range(4)]
            for kt in range(NKT):
                pss = [ps_tile("sT0"), ps_tile("sT1")]
                for qbh in range(2):
                    for hh in range(2):
                        r0 = hh * 64
                        nc.tensor.matmul(
                            pss[hh][:, qbh * 512:(qbh + 1) * 512],
                            lhsT=(kt_t[jt][r0:r0 + 64, kt * 128:(kt + 1) * 128]),
                            rhs=(qt_t[jt][r0:r0 + 64,
                                          q0 + qbh * 512:q0 + (qbh + 1) * 512]),
                            start=True,
                            stop=True,
                        )
                for hh in range(2):
                    et = expp.tile([128, 1024], F32R, tag="expT", name="expT",
                                   bufs=3)
                    nc.scalar.activation(et[:], pss[hh][:], EXP)
                    for qbh in range(2):
                        nc.tensor.matmul(
                            avs[hh * 2 + qbh][0:DK + 1, :],
                            lhsT=(v_t[kt][:, 2 * jt + hh, :]),
                            rhs=(et[:, qbh * 512:(qbh + 1) * 512]),
                            start=(kt == 0),
                            stop=(kt == NKT - 1),
                        )
            for hh in range(2):
                for qbh in range(2):
                    av = avs[hh * 2 + qbh]
                    qoff = q0 + qbh * 512
                    avsb = smalls.tile([128, 512], F32, tag="avsb", name="avsb",
                                       bufs=4)
                    nc.vector.tensor_copy(out=avsb[0:DK + 1, :],
                                          in_=av[0:DK + 1, :])
                    av = avsb
                    rc = smalls.tile([128, 512], F32, tag="rcp", name="rcp")
                    nc.vector.reciprocal(rc[DK:DK + 1, :], av[DK:DK + 1, :])
                    rdram = dramp.tile([512], F32, tag="rdram", name="rdram",
                                       bufs=4)
                    nc.sync.dma_start(out=rdram[None, :], in_=rc[DK:DK + 1, :])
                    rb = smalls.tile([128, 512], F32, tag="rbc", name="rbc")
                    nc.sync.dma_start(out=rb[0:DK, :],
                                      in_=_bcast_rows(rdram[None, :], DK))
                    op = smalls.tile([128, 512], F32R, tag="ocp", name="ocp")
                    nc.vector.tensor_mul(out=op[0:DK, :], in0=av[0:DK, :],
                                         in1=rb[0:DK, :])
                    nc.sync.dma_start(
                        out=oc_dram[jt * 128 + hh * 64:jt * 128 + (hh + 1) * 64,
                                    qoff:qoff + 512],
                        in_=op[0:DK, :],
                    )

    # ---- output projection (partial y; host adds the pair + bias) ----
    wo_sb = wpool.tile([128, NJT, D], F32R, tag="w", name="w_o")
    nc.gpsimd.dma_start(out=wo_sb, in_=wo.rearrange("(n p) j -> p n j", p=128))
    for qt in range(NQT):
        ocl = [ocstage.tile([128, 128], F32R, tag="ocl", name=f"ocl{jt}")
               for jt in range(NJT)]
        for jt in range(NJT):
            nc.sync.dma_start(
                out=ocl[jt],
                in_=oc_dram[jt * 128:(jt + 1) * 128, qt * 128:(qt + 1) * 128],
            )
        py = [pb(f"py{nb}") for nb in range(2)]
        for jt in range(NJT):
            for nb in range(2):
                nc.tensor.matmul(
                    py[nb][:],
                    lhsT=(ocl[jt][:]),
                    rhs=(wo_sb[:, jt, nb * 512:(nb + 1) * 512]),
                    start=(jt == 0),
                    stop=(jt == NJT - 1),
                )
        ys = ystage.tile([128, D], F32, tag="y", name="ys")
        for nb in range(2):
            nc.vector.tensor_copy(out=ys[:, nb * 512:(nb + 1) * 512], in_=py[nb][:])
        nc.sync.dma_start(out=y[qt * 128:(qt + 1) * 128, :], in_=ys[:])


_CACHE = {}


def _build():
    if "nc" in _CACHE:
        return _CACHE["nc"]
    from contextlib import ExitStack

    nc = bacc.Bacc("TRN2", target_bir_lowering=False, debug=False,
                   num_devices=NCORES)
    with tile.TileContext(nc) as tc:
        with ExitStack() as ctx:
            _emit(nc, tc, ctx)
    nc.compile()
    _CACHE["nc"] = nc
    return nc


def make_in_maps(query, key, value, Wq, bq, Wk, bk, Wv, bv, Wo, bo):
    arrs = [np.ascontiguousarray(np.asarray(a, dtype=np.float32))
            for a in (query, key, value, Wq, bq, Wk, bk, Wv, bv, Wo, bo)]
    query, key, value, Wq, bq, Wk, bk, Wv, bv, Wo, bo = arrs
    scale = np.float32(1.0 / np.sqrt(DK))
    in_maps = []
    for c in range(NCORES):
        b, hh = divmod(c, 2)
        js = slice(hh * DH, (hh + 1) * DH)
        in_maps.append({
            "xq": query[b],
            "xk": key[b],
            "xv": value[b],
            "wq": np.ascontiguousarray(Wq[:, js] * scale),
            "bq": np.ascontiguousarray(bq[js] * scale),
            "wk": np.ascontiguousarray(Wk[:, js]),
            "bk": np.ascontiguousarray(bk[js]),
            "wv": np.ascontiguousarray(Wv[:, js]),
            "bv": np.ascontiguousarray(bv[js]),
            "wo": np.ascontiguousarray(Wo[js, :]),
        })
    return in_maps


LAST_RESULTS = None


def kernel(query, key, value, Wq, bq, Wk, bk, Wv, bv, Wo, bo):
    global LAST_RESULTS
    import os
    from concourse.bass_utils import run_bass_kernel_spmd

    nc = _build()
    in_maps = make_in_maps(query, key, value, Wq, bq, Wk, bk, Wv, bv, Wo, bo)
    trace = bool(int(os.environ.get("KERNEL_TRACE", "0")))
    res = run_bass_kernel_spmd(nc, in_maps, list(range(NCORES)), trace=trace)
    LAST_RESULTS = res
    bo32 = np.asarray(bo, dtype=np.float32)
    out = np.empty((B, S, D), dtype=np.float32)
    for b in range(B):
        out[b] = res.results[2 * b]["y"] + res.results[2 * b + 1]["y"] + bo32
    return out
